# revision 2
# baseline (speedup 1.0000x reference)
"""Trainium2 Bass kernel for nn_MultiHeadCrossAttention (B=4, T=1024, E=1024, H=16).

Sharding: the computation splits into 8 fully independent shards with zero
cross-core communication: (output stream s, batch b) for s in {1,2}, b in 0..3.
Stream-1 output xo@Wout1 needs K,V from x and Q from y; stream-2 the reverse.
Core c<4 computes stream-1 batch c; core c>=4 computes stream-2 batch c-4.

Per-core kernel (all activations kept transposed, feature-on-partition):
  Q^T = Wq^T.T @ B^T, K^T = Wk^T.T @ A^T   (fp32r matmuls, K=1024)
  V   = A^T.T @ Wv^T                        (natural layout, bf16 store)
  per head pair (2m, 2m+1), row-tiled K=64 matmuls:
    S^T[j,i] = K^T.T @ Q^T;  P^T = exp(S^T/8) on ACT (bf16)
    O'^T = V.T @ P^T (col-tiled M=64 pairs) ; rowsums via M=1 ones-matmuls
    recip = 1/rowsum (DVE); broadcast via K=1 ones-matmul; O^T = O'^T * bcast
  Z^T = Wout^T.T @ O^T  (fp32r, accumulate over head chunks)
Host side pre-transposes/groups weights and activations, and re-transposes
the per-core outputs.
"""

import os
import sys
import time

sys.path.insert(0, "/opt/trn_rl_repo")

import numpy as np
import ml_dtypes
from contextlib import ExitStack

import concourse.bass as bass
import concourse.mybir as mybir
import concourse.tile as tile
from concourse import bacc
from concourse import bass_utils

B, T, E, H = 4, 1024, 1024, 16
D = E // H            # 64
NC = E // 128         # 8 chunks of 128
NIC = T // 512        # 2 free-dim chunks of 512
N_CORES = 8

F32R = mybir.dt.float32r
F32 = mybir.dt.float32
BF16 = mybir.dt.bfloat16
F16 = mybir.dt.bfloat16

_NC_CACHE = {}
LAST_RESULTS = {}

_SELBC = np.zeros((33, 128), np.float32)
_SELBC[0, 0:64] = 1.0
_SELBC[32, 64:128] = 1.0


KPHASE = os.environ.get("KPHASE", "PAZ")
KSKIP = set(os.environ.get("KSKIP", "").split(","))



def _build():
    KREP = int(os.environ.get("KREP", "1"))
    nc = bacc.Bacc("TRN2", target_bir_lowering=False, debug=False,
                   enable_asserts=False, num_devices=N_CORES)
    a_t = nc.dram_tensor("a_t", (E, T), F16, kind="ExternalInput").ap()
    b_t = nc.dram_tensor("b_t", (E, T), F16, kind="ExternalInput").ap()
    wq_t = nc.dram_tensor("wq_t", (E, E), F16, kind="ExternalInput").ap()
    wk_t = nc.dram_tensor("wk_t", (E, E), F16, kind="ExternalInput").ap()
    wv_t = nc.dram_tensor("wv_t", (E, E), F16, kind="ExternalInput").ap()
    wout_t = nc.dram_tensor("wout_t", (E, E), F16, kind="ExternalInput").ap()
    selbc_d = nc.dram_tensor("selbc", (33, 128), F16, kind="ExternalInput").ap()
    z_t = nc.dram_tensor("z_t", (E, T), F32, kind="ExternalOutput").ap()

    with tile.TileContext(nc) as tc, ExitStack() as ctx:
        # long-lived pools
        qkv_pool = ctx.enter_context(tc.tile_pool(name="qkv", bufs=1))
        const_pool = ctx.enter_context(tc.tile_pool(name="const", bufs=1))

        qt = qkv_pool.tile([128, NC, T], F16, tag="qt")
        kt = qkv_pool.tile([128, NC, T], F16, tag="kt")
        v = qkv_pool.tile([128, NC, H * (D + 1)], F16, tag="v")

        for _m in range(NC):
            nc.vector.memset(
                v[:, _m, :].rearrange("p (h x) -> p h x", x=D + 1)[:, :, D:D + 1], 1.0)
        if KSKIP & {"proj"}:
            for m in range(NC):
                nc.vector.memset(qt[:, m, :], 0.25)
                nc.vector.memset(kt[:, m, :], 0.25)
                nc.vector.memset(v[:, m, :], 0.25)
        selbc = const_pool.tile([33, 128], F16, tag="selbc")
        nc.sync.dma_start(selbc[:], selbc_d)

        # ---------------- Phase P: projections ----------------
        for _rep in range(KREP):
            _build_body(nc, tc, ctx, locals())
    nc.compile()
    return nc


def _build_body(nc, tc, ctx, env):
    qt, kt, v = env["qt"], env["kt"], env["v"]
    selbc = env["selbc"]
    a_t, b_t = env["a_t"], env["b_t"]
    wq_t, wk_t, wv_t, wout_t = env["wq_t"], env["wk_t"], env["wv_t"], env["wout_t"]
    z_t = env["z_t"]
    if True:
        with tc.tile_pool(name="acts", bufs=1) as acts, \
             tc.tile_pool(name="pps", bufs=3, space="PSUM") as pps:
            at_sb = acts.tile([128, NC, T], F16, tag="at")
            bt_sb = acts.tile([128, NC, T], F16, tag="bt")
            wv_sb = acts.tile([128, NC, E], F16, tag="wv")
            wq_sb = acts.tile([128, NC, E], F16, tag="wq")
            wk_sb = acts.tile([128, NC, E], F16, tag="wk")
            # issue order matters: Q^T-proj (bt, wq) starts first
            for c in range(NC):
                nc.sync.dma_start(bt_sb[:, c, :], b_t[c * 128:(c + 1) * 128, :])
                nc.sync.dma_start(wq_sb[:, c, :], wq_t[c * 128:(c + 1) * 128, :])
            for c in range(NC):
                nc.sync.dma_start(at_sb[:, c, :], a_t[c * 128:(c + 1) * 128, :])
                nc.sync.dma_start(wk_sb[:, c, :], wk_t[c * 128:(c + 1) * 128, :])
            for c in range(NC):
                nc.sync.dma_start(wv_sb[:, c, :], wv_t[c * 128:(c + 1) * 128, :])

            # Q^T and K^T: out[dh-chunk m][t] = sum_e w[e, dh] * act[e, t]
            # weight column-blocks streamed per m (each block used exactly once)
            for (w_sb, act_sb, out_sb) in (
                () if "proj" in KSKIP else (
                (wq_sb, bt_sb, qt),
                (wk_sb, at_sb, kt),
            )):
                for m in range(NC):
                    ps = pps.tile([128, T], F32, tag="pp")
                    for e in range(NC):
                        for ic in range(NIC):
                            nc.tensor.matmul(
                                ps[:, bass.ts(ic, 512)],
                                w_sb[:, e, bass.ts(m, 128)],
                                act_sb[:, e, bass.ts(ic, 512)],
                                start=(e == 0), stop=(e == NC - 1))
                    nc.scalar.copy(out_sb[:, m, :], ps[:])
            # V natural: out[j-chunk][dv] = sum_e at[e, j] * wv[e, dv]
            for m in range(NC) if "proj" not in KSKIP else ():
                ps = pps.tile([128, T], F32, tag="pp")
                for e in range(NC):
                    for ic in range(NIC):
                        nc.tensor.matmul(
                            ps[:, bass.ts(ic, 512)],
                            at_sb[:, e, bass.ts(m, 128)],
                            wv_sb[:, e, bass.ts(ic, 512)],
                            start=(e == 0), stop=(e == NC - 1))
                nc.scalar.copy(
                    v[:, m, :].rearrange("p (h x) -> p h x", x=D + 1)[:, :, 0:D],
                    ps[:].rearrange("p (h x) -> p h x", x=D))

        if KPHASE == "P":
            with tc.tile_pool(name="zdbg", bufs=2) as zdbgp:
                for cc in range(NC):
                    zdbg = zdbgp.tile([128, T], F32, tag="zdbg")
                    nc.vector.tensor_copy(zdbg[:], qt[:, cc, :])
                    nc.sync.dma_start(z_t[cc * 128:(cc + 1) * 128, :], zdbg[:])
            return

        # ---------------- Phase A: attention per head pair ----------------
        rep_ctx = ctx.enter_context(ExitStack())
        shps = rep_ctx.enter_context(tc.tile_pool(name="shps", bufs=2, space="PSUM"))
        ot_pool = rep_ctx.enter_context(tc.tile_pool(name="ot", bufs=1))
        ot = ot_pool.tile([128, NC, T], F16, tag="ot")
        with tc.tile_pool(name="pt", bufs=4) as ptp, \
             tc.tile_pool(name="nrm", bufs=4) as nrm, \
             tc.tile_pool(name="nrm8", bufs=8) as nrm8, \
             tc.tile_pool(name="ops", bufs=2, space="PSUM") as ops:
            for m in range(NC):
                ptA = ptp.tile([128, NC, T], F16, tag="pt")
                ptB = ptp.tile([128, NC, T], F16, tag="pt")
                if "sexp" in KSKIP:
                    nc.vector.memset(ptA[:], 1.0)
                    nc.vector.memset(ptB[:], 1.0)
                for jc in range(NC) if "sexp" not in KSKIP else ():
                    ps_s = shps.tile([128, T], F32, tag="big")
                    ps_sB = shps.tile([128, T], F32, tag="big")
                    for ic in range(NIC):
                        nc.tensor.matmul(
                            ps_s[:, bass.ts(ic, 512)],
                            kt[0:64, m, bass.ts(jc, 128)],
                            qt[0:64, m, bass.ts(ic, 512)],
                            start=True, stop=True)
                        nc.tensor.matmul(
                            ps_sB[:, bass.ts(ic, 512)],
                            kt[64:128, m, bass.ts(jc, 128)],
                            qt[64:128, m, bass.ts(ic, 512)],
                            start=True, stop=True, tile_position=(64, 0))
                    nc.scalar.activation(ptA[:, jc, :], ps_s[:],
                                         mybir.ActivationFunctionType.Exp, scale=0.125)
                    nc.scalar.activation(ptB[:, jc, :], ps_sB[:],
                                         mybir.ActivationFunctionType.Exp, scale=0.125)

                if "ovr" in KSKIP:
                    nc.vector.memset(ot[:, m, :], 0.25)
                    continue
                ps_oA = ops.tile([65, T], F32, tag="o")
                ps_oB = ops.tile([65, T], F32, tag="o")
                hA, hB = 2 * m, 2 * m + 1
                for jc in range(NC):
                    st = dict(start=(jc == 0), stop=(jc == NC - 1))
                    for ic in range(NIC):
                        s_ic = bass.ts(ic, 512)
                        nc.tensor.matmul(ps_oA[:, s_ic], v[:, jc, bass.ts(hA, D + 1)],
                                         ptA[:, jc, s_ic], **st)
                        nc.tensor.matmul(ps_oB[:, s_ic], v[:, jc, bass.ts(hB, D + 1)],
                                         ptB[:, jc, s_ic], **st)

                recip2 = nrm.tile([33, T], F16, tag="recip2")
                nc.vector.memset(recip2[:], 0.0)
                with nc.allow_low_precision(reason="recip feeds fp16 bc matmul"):
                    nc.vector.reciprocal(recip2[0:1, :], ps_oA[64:65, :])
                    nc.vector.reciprocal(recip2[32:33, :], ps_oB[64:65, :])
                ps_bcA = shps.tile([64, T], F32, tag="big")
                ps_bcB = shps.tile([64, T], F32, tag="big")
                for ic in range(NIC):
                    s_ic = bass.ts(ic, 512)
                    nc.tensor.matmul(ps_bcA[:, s_ic], selbc[:, 0:64], recip2[:, s_ic],
                                     start=True, stop=True)
                    nc.tensor.matmul(ps_bcB[:, s_ic], selbc[:, 64:128], recip2[:, s_ic],
                                     start=True, stop=True)
                bcA = nrm.tile([64, T], F32, tag="bcA")
                bcB = nrm.tile([64, T], F32, tag="bcB")
                nc.scalar.copy(bcA[:], ps_bcA[:])
                nc.scalar.copy(bcB[:], ps_bcB[:])
                with nc.allow_low_precision(reason="O^T fp16 feeds fp16 out-proj"):
                    nc.vector.tensor_mul(ot[0:64, m, :], ps_oA[0:64, :], bcA[:])
                    nc.vector.tensor_mul(ot[64:128, m, :], ps_oB[0:64, :], bcB[:])

        if KPHASE == "PA":
            with tc.tile_pool(name="zdbg", bufs=2) as zdbgp:
                for cc in range(NC):
                    zdbg = zdbgp.tile([128, T], F32, tag="zdbg")
                    nc.vector.tensor_copy(zdbg[:], ot[:, cc, :])
                    nc.sync.dma_start(z_t[cc * 128:(cc + 1) * 128, :], zdbg[:])
            return

        # ---------------- Phase Z: out-projection ----------------
        with tc.tile_pool(name="wout", bufs=1) as woutp, \
             tc.tile_pool(name="zsb", bufs=2) as zsbp, \
             tc.tile_pool(name="zps", bufs=2, space="PSUM") as zps:
            wo = woutp.tile([128, NC, E], F16, tag="wo")
            for c in range(NC):
                nc.sync.dma_start(wo[:, c, :], wout_t[c * 128:(c + 1) * 128, :])
            for cc in range(NC):
                ps = zps.tile([128, T], F32, tag="z")
                for m in range(NC):
                    for ic in range(NIC):
                        nc.tensor.matmul(
                            ps[:, bass.ts(ic, 512)],
                            wo[:, m, bass.ts(cc, 128)],
                            ot[:, m, bass.ts(ic, 512)],
                            start=(m == 0), stop=(m == NC - 1))
                zsb = zsbp.tile([128, T], F32, tag="zsb")
                nc.scalar.copy(zsb[:], ps[:])
                nc.sync.dma_start(z_t[cc * 128:(cc + 1) * 128, :], zsb[:])
        rep_ctx.close()


def _group_w(wqkv, k):
    """Rows of Wqkv (3E, E) for q/k/v (k=0/1/2), grouped head-major.

    Row index layout: r = di*(3H) + k*H + h  ->  grouped[h*D+di, :].
    """
    w = np.asarray(wqkv, dtype=np.float32).reshape(D, 3, H, E)[:, k]   # [di, h, e]
    return np.ascontiguousarray(w.transpose(1, 0, 2).reshape(E, E))    # [h*D+di, e]


def kernel(x, y, Wqkv1, Wqkv2, Wout1, Wout2):
    x = np.asarray(x, dtype=np.float32)
    y = np.asarray(y, dtype=np.float32)

    if "nc" not in _NC_CACHE:
        _NC_CACHE["nc"] = _build()
    nc = _NC_CACHE["nc"]

    # weight prep (host): grouped + transposed (fp16 on-device dtype)
    wq1_t = np.ascontiguousarray(_group_w(Wqkv1, 0).T)
    wk1_t = np.ascontiguousarray(_group_w(Wqkv1, 1).T)
    wv1_t = np.ascontiguousarray(_group_w(Wqkv1, 2).T)
    wq2_t = np.ascontiguousarray(_group_w(Wqkv2, 0).T)
    wk2_t = np.ascontiguousarray(_group_w(Wqkv2, 1).T)
    wv2_t = np.ascontiguousarray(_group_w(Wqkv2, 2).T)
    wout1_t = np.ascontiguousarray(np.asarray(Wout1, dtype=np.float32).T)
    wout2_t = np.ascontiguousarray(np.asarray(Wout2, dtype=np.float32).T)

    in_maps = []
    for c in range(N_CORES):
        s, b = divmod(c, B)
        if s == 0:
            # stream-1 output: K,V from x via Wqkv1; Q from y via Wqkv2
            a_t, b_t = x[b].T, y[b].T
            wq, wk, wv, wo = wq2_t, wk1_t, wv1_t, wout1_t
        else:
            a_t, b_t = y[b].T, x[b].T
            wq, wk, wv, wo = wq1_t, wk2_t, wv2_t, wout2_t
        in_maps.append({
            "a_t": np.ascontiguousarray(a_t).astype(ml_dtypes.bfloat16),
            "b_t": np.ascontiguousarray(b_t).astype(ml_dtypes.bfloat16),
            "wq_t": wq.astype(ml_dtypes.bfloat16), "wk_t": wk.astype(ml_dtypes.bfloat16),
            "wv_t": wv.astype(ml_dtypes.bfloat16), "wout_t": wo.astype(ml_dtypes.bfloat16),
            "selbc": _SELBC.astype(ml_dtypes.bfloat16),
        })

    trace = os.environ.get("BASS_KERNEL_TRACE", "0") == "1"
    if trace:
        try:
            from antenv.axon_hooks import get_axon_ntff_profile_hook  # noqa: F401
        except ImportError:
            trace = False
    ncores = int(os.environ.get("KCORES", str(N_CORES)))
    r = bass_utils.run_bass_kernel_spmd(nc, in_maps[:ncores], core_ids=list(range(ncores)),
                                        trace=trace)
    LAST_RESULTS["exec_time_ns"] = r.exec_time_ns
    LAST_RESULTS["profile_json"] = r.profile_json

    out1 = np.stack([r.results[b]["z_t"].T for b in range(B)]).astype(np.float32)
    out2 = np.stack([r.results[B + b]["z_t"].T for b in range(B)]).astype(np.float32)
    return out1, out2



# revision 20
# speedup vs baseline: 1.5085x; 1.5085x over previous
"""Trainium2 Bass kernel for nn_MultiHeadCrossAttention (B=4, T=1024, E=1024, H=16).

Sharding: 8 fully independent shards, zero cross-core communication:
(output stream s, batch b) for s in {1,2}, b in 0..3. Stream-1 output
xo@Wout1 needs K,V from x and Q from y; stream-2 the reverse.

Per-core kernel (activations transposed, feature-on-partition):
  Preamble: V = A^T.T @ Wv^T (natural, with ones column per head for the
  rowsum trick); Q^T/K^T chunks 0,1.
  m-loop over 8 head pairs (hA=2m, hB=2m+1), 16 slots (jc, ic) each:
    S^T pair via two concurrent K=64 matmuls (tile_position row split)
    P = exp(S/8) on ACT into SBUF fp16 (A|B merged per slot)
    O'^T accumulation (M=65 incl. ones row -> rowsum) lagged one slot
    Q^T/K^T projection chunk m+2 interleaved (2 matmuls per slot)
    normalization of pair m-1 lagged: reciprocal_approx_fast + gpsimd
    partition_broadcast + fp16 multiply into ot
  Tail: Z^T = Wout^T.T @ O^T accumulated over head pairs, DMA out.
"""

import os
import sys

sys.path.insert(0, "/opt/trn_rl_repo")

import numpy as np
import ml_dtypes
from contextlib import ExitStack

import concourse.bass as bass
import concourse.mybir as mybir
import concourse.tile as tile
from concourse import bacc
from concourse import bass_utils

B, T, E, H = 4, 1024, 1024, 16
D = E // H            # 64
NC = E // 128         # 8 chunks of 128
N_CORES = 8

F32 = mybir.dt.float32
F16 = mybir.dt.float16

_NC_CACHE = {}
LAST_RESULTS = {}
_KDBG = os.environ.get("KDBG", "0") == "1"
_KDBG2 = os.environ.get("KDBG", "0") == "2"
_DBG_TILES = {}


def _build():
    nc = bacc.Bacc("TRN2", target_bir_lowering=False, debug=False,
                   enable_asserts=False, num_devices=N_CORES)
    a_t = nc.dram_tensor("a_t", (E, T), F16, kind="ExternalInput").ap()
    b_t = nc.dram_tensor("b_t", (E, T), F16, kind="ExternalInput").ap()
    wq_t = nc.dram_tensor("wq_t", (E, E), F16, kind="ExternalInput").ap()
    wk_t = nc.dram_tensor("wk_t", (E, E), F16, kind="ExternalInput").ap()
    wv_t = nc.dram_tensor("wv_t", (E, E), F16, kind="ExternalInput").ap()
    wout_t = nc.dram_tensor("wout_t", (E, E), F16, kind="ExternalInput").ap()
    z_t = nc.dram_tensor("z_t", (E, T), F32, kind="ExternalOutput").ap()

    EXP = mybir.ActivationFunctionType.Exp

    with tile.TileContext(nc) as tc, ExitStack() as ctx:
        persist = ctx.enter_context(tc.tile_pool(name="persist", bufs=1))
        qt = persist.tile([128, NC, T], F16, tag="qt")
        kt = persist.tile([128, NC, T], F16, tag="kt")
        v = persist.tile([128, NC, H * (D + 1)], F16, tag="v")
        ot = persist.tile([128, NC, T], F16, tag="ot")
        wo_sb = None if _KDBG else persist.tile([128, NC, E], F16, tag="wo", name="wo_sb")

        for mch in range(NC):
            nc.vector.memset(
                v[:, mch, :].rearrange("p (h x) -> p h x", x=D + 1)[:, :, D:D + 1],
                1.0)
        if _KDBG:
            _DBG_TILES["zd"] = persist.tile([128, 6, T], F32, tag="zd", name="zd")
            nc.vector.memset(_DBG_TILES["zd"][:], 0.0)

        acts = ctx.enter_context(tc.tile_pool(name="acts", bufs=1))
        at_sb = acts.tile([128, NC, T], F16, tag="at")
        bt_sb = acts.tile([128, NC, T], F16, tag="bt")
        wq_sb = acts.tile([128, NC, E], F16, tag="wq")
        wk_sb = acts.tile([128, NC, E], F16, tag="wk")

        # ---------------- Preamble: V proj + QK chunks 0,1 ----------------
        with tc.tile_pool(name="wvp", bufs=1) as wvp, \
             tc.tile_pool(name="pps", bufs=2, space="PSUM") as pps:
            wv_sb = wvp.tile([128, NC, E], F16, tag="wv")
            # DMA priority: (at, wv) pairs first so V proj starts ASAP
            for e in range(NC):
                nc.sync.dma_start(at_sb[:, e, :], a_t[e * 128:(e + 1) * 128, :])
                nc.sync.dma_start(wv_sb[:, e, :], wv_t[e * 128:(e + 1) * 128, :])
            for e in range(NC):
                nc.sync.dma_start(bt_sb[:, e, :], b_t[e * 128:(e + 1) * 128, :])
                nc.sync.dma_start(wq_sb[:, e, :], wq_t[e * 128:(e + 1) * 128, :])
            for e in range(NC):
                nc.sync.dma_start(wk_sb[:, e, :], wk_t[e * 128:(e + 1) * 128, :])
            for e in range(NC) if not _KDBG else ():
                nc.sync.dma_start(wo_sb[:, e, :], wout_t[e * 128:(e + 1) * 128, :])

            # V natural: out[j-chunk m][h*d] = sum_e at[e, j].T @ wv[e, hd]
            for mch in range(NC):
                ps = pps.tile([128, T], F32, tag="pp")
                for e in range(NC):
                    for ic in range(2):
                        nc.tensor.matmul(
                            ps[:, bass.ts(ic, 512)],
                            at_sb[:, e, bass.ts(mch, 128)],
                            wv_sb[:, e, bass.ts(ic, 512)],
                            start=(e == 0), stop=(e == NC - 1))
                with nc.allow_low_precision(reason="V fp16 feeds fp16 matmul"):
                    nc.vector.tensor_copy(
                        v[:, mch, :].rearrange("p (h x) -> p h x", x=D + 1)[:, :, 0:D],
                        ps[:].rearrange("p (h d) -> p h d", d=D))

            # Q^T/K^T chunks 0 and 1
            for ch in (0, 1):
                for (w_sb, act_sb, out_sb) in ((wq_sb, bt_sb, qt), (wk_sb, at_sb, kt)):
                    ps = pps.tile([128, T], F32, tag="pp")
                    for e in range(NC):
                        for ic in range(2):
                            nc.tensor.matmul(
                                ps[:, bass.ts(ic, 512)],
                                w_sb[:, e, bass.ts(ch, 128)],
                                act_sb[:, e, bass.ts(ic, 512)],
                                start=(e == 0), stop=(e == NC - 1))
                    with nc.allow_low_precision(reason="QK fp16 feeds fp16 matmul"):
                        nc.vector.tensor_copy(out_sb[:, ch, :], ps[:])

        # ---------------- m-loop: attention over 8 head pairs ----------------
        with tc.tile_pool(name="sps", bufs=1, space="PSUM") as sps_pool, \
             tc.tile_pool(name="ops", bufs=1, space="PSUM") as ops, \
             tc.tile_pool(name="pjp", bufs=1, space="PSUM") as pjp, \
             tc.tile_pool(name="ptp", bufs=4) as ptp, \
             tc.tile_pool(name="oup", bufs=2) as oup, \
             tc.tile_pool(name="nrm", bufs=2) as nrm:

            SLOTS = [(jc, ic) for jc in range(NC) for ic in range(2)]
            pending_o = None   # (pt_tile, jc, ic, ps_oA, ps_oB, hA, hB)
            pending_norm = None  # (m, ouA, ouB, rs)

            def issue_o(po):
                pt_prev, jc, ic, psA, psB, hA, hB = po
                st = dict(start=(jc == 0), stop=(jc == NC - 1))
                nc.tensor.matmul(psA[:, bass.ts(ic, 512)],
                                 v[:, jc, hA * (D + 1):(hA + 1) * (D + 1)],
                                 pt_prev[:, 0:512], **st)
                nc.tensor.matmul(psB[:, bass.ts(ic, 512)],
                                 v[:, jc, hB * (D + 1):(hB + 1) * (D + 1)],
                                 pt_prev[:, 512:1024], **st)

            def issue_norm(pn):
                mm, ouA, ouB, rsA, rsB = pn
                rrA = nrm.tile([1, T], F32, tag="rrA", bufs=1)
                rrB = nrm.tile([1, T], F32, tag="rrB", bufs=1)
                nc.vector.reciprocal_approx_fast(rrA[:], rsA[:])
                nc.vector.reciprocal_approx_fast(rrB[:], rsB[:])
                rrhA = nrm.tile([1, T], F16, tag="rrhA", bufs=1)
                rrhB = nrm.tile([1, T], F16, tag="rrhB", bufs=1)
                with nc.allow_low_precision(reason="recip feeds fp16 multiply"):
                    nc.vector.tensor_copy(rrhA[:], rrA[:])
                    nc.vector.tensor_copy(rrhB[:], rrB[:])
                bcA = nrm.tile([64, T], F16, tag="bcA", bufs=1)
                bcB = nrm.tile([64, T], F16, tag="bcB", bufs=1)
                nc.gpsimd.partition_broadcast(bcA[:], rrhA[:])
                nc.gpsimd.partition_broadcast(bcB[:], rrhB[:])
                with nc.allow_low_precision(reason="O^T fp16 feeds fp16 out-proj"):
                    nc.vector.tensor_mul(ot[0:64, mm, :], ouA[:], bcA[:])
                    nc.vector.tensor_mul(ot[64:128, mm, :], ouB[:], bcB[:])
                if _KDBG and mm == 0:
                    zd = _DBG_TILES["zd"]
                    nc.vector.tensor_copy(zd[0:64, 0, :], ouB[:])
                    nc.vector.tensor_copy(zd[0:64, 1, :], bcB[:])
                    nc.vector.tensor_copy(zd[0:1, 2, :], rsB[:])
                    nc.vector.tensor_copy(zd[32:33, 2, :], rrB[:])
                    nc.vector.tensor_copy(zd[64:65, 2, :], rrhB[:])

            for m in range(NC):
                hA, hB = 2 * m, 2 * m + 1
                ps_oA = ops.tile([D + 1, T], F32, tag="oA")
                ps_oB = ops.tile([D + 1, T], F32, tag="oB")
                if pending_norm is not None:
                    issue_norm(pending_norm)
                    pending_norm = None

                pj = None
                for s, (jc, ic) in enumerate(SLOTS):
                    # S pair: two concurrent K=64 matmuls (row-split)
                    sps = sps_pool.tile([128, 1024], F32, tag="s")
                    nc.tensor.matmul(
                        sps[:, 0:512],
                        kt[0:64, m, bass.ts(jc, 128)],
                        qt[0:64, m, bass.ts(ic, 512)],
                        start=True, stop=True)
                    nc.tensor.matmul(
                        sps[:, 512:1024],
                        kt[64:128, m, bass.ts(jc, 128)],
                        qt[64:128, m, bass.ts(ic, 512)],
                        start=True, stop=True, tile_position=(64, 0))
                    pt_t = ptp.tile([128, 1024], F16, tag="pt")
                    nc.scalar.activation(pt_t[:], sps[:], EXP, scale=0.125)
                    if _KDBG and m == 0 and s == 0:
                        nc.vector.tensor_copy(_DBG_TILES["zd"][:, 4, :], pt_t[:])

                    if pending_o is not None:
                        issue_o(pending_o)
                    pending_o = (pt_t, jc, ic, ps_oA, ps_oB, hA, hB)

                    # interleaved Q^T/K^T projection for chunk m+2
                    if m < NC - 2:
                        ch = m + 2
                        if s < 8:
                            e = s
                            if pj is None:
                                pj = pjp.tile([128, T], F32, tag="pj")
                            for icc in range(2):
                                nc.tensor.matmul(
                                    pj[:, bass.ts(icc, 512)],
                                    wq_sb[:, e, bass.ts(ch, 128)],
                                    bt_sb[:, e, bass.ts(icc, 512)],
                                    start=(e == 0), stop=(e == NC - 1))
                            if s == 7:
                                with nc.allow_low_precision(reason="QK fp16"):
                                    nc.vector.tensor_copy(qt[:, ch, :], pj[:])
                                pj = None
                        else:
                            e = s - 8
                            if pj is None:
                                pj = pjp.tile([128, T], F32, tag="pj")
                            for icc in range(2):
                                nc.tensor.matmul(
                                    pj[:, bass.ts(icc, 512)],
                                    wk_sb[:, e, bass.ts(ch, 128)],
                                    at_sb[:, e, bass.ts(icc, 512)],
                                    start=(e == 0), stop=(e == NC - 1))
                            if s == 15:
                                with nc.allow_low_precision(reason="QK fp16"):
                                    nc.vector.tensor_copy(kt[:, ch, :], pj[:])
                                pj = None

                # flush last O slot of this pair, then evacuate O' + rowsums
                issue_o(pending_o)
                pending_o = None
                ouA = oup.tile([D, T], F16, tag="ouA")
                ouB = oup.tile([D, T], F16, tag="ouB")
                rsA = nrm.tile([1, T], F32, tag="rsA", bufs=1)
                rsB = nrm.tile([1, T], F32, tag="rsB", bufs=1)
                with nc.allow_low_precision(reason="O' fp16 feeds fp16 multiply"):
                    nc.vector.tensor_copy(ouA[:], ps_oA[0:D, :])
                    nc.vector.tensor_copy(ouB[:], ps_oB[0:D, :])
                nc.vector.tensor_copy(rsA[:], ps_oA[D:D + 1, :])
                nc.vector.tensor_copy(rsB[:], ps_oB[D:D + 1, :])
                pending_norm = (m, ouA, ouB, rsA, rsB)

            issue_norm(pending_norm)
            pending_norm = None

        if _KDBG:
            with tc.tile_pool(name="zdbg2", bufs=1) as zp2:
                zd = _DBG_TILES["zd"]
                nc.vector.tensor_copy(zd[0:64, 3, :], ot[0:64, 0, :])
                nc.vector.tensor_copy(zd[64:128, 3, :], ot[64:128, 0, :])
                nc.vector.tensor_copy(zd[:, 5, :], qt[:, 2, :])
                for cc in range(6):
                    nc.sync.dma_start(z_t[cc * 128:(cc + 1) * 128, :],
                                      zd[:, cc, :])

        if _KDBG2:
            with tc.tile_pool(name="zdbg3", bufs=2) as zp3:
                for mm in range(NC):
                    zc = zp3.tile([128, T], F32, tag="zc", name="zc")
                    nc.vector.tensor_copy(zc[:], ot[:, mm, :])
                    nc.sync.dma_start(z_t[mm * 128:(mm + 1) * 128, :], zc[:])

        # ---------------- Z: out-projection ----------------
        if not _KDBG and not _KDBG2:
          with tc.tile_pool(name="zps", bufs=2, space="PSUM") as zps, \
             tc.tile_pool(name="zsb", bufs=2) as zsbp:
            for cc in range(NC):
                ps = zps.tile([128, T], F32, tag="z")
                for mm in range(NC):
                    for ic in range(2):
                        nc.tensor.matmul(
                            ps[:, bass.ts(ic, 512)],
                            wo_sb[:, mm, bass.ts(cc, 128)],
                            ot[:, mm, bass.ts(ic, 512)],
                            start=(mm == 0), stop=(mm == NC - 1))
                zsb = zsbp.tile([128, T], F32, tag="zsb")
                nc.vector.tensor_copy(zsb[:], ps[:])
                nc.sync.dma_start(z_t[cc * 128:(cc + 1) * 128, :], zsb[:])
    nc.compile()
    return nc


def _group_w(wqkv, k):
    """Rows of Wqkv (3E, E) for q/k/v (k=0/1/2), grouped head-major.

    Row index layout: r = di*(3H) + k*H + h  ->  grouped[h*D+di, :].
    """
    w = np.asarray(wqkv, dtype=np.float32).reshape(D, 3, H, E)[:, k]   # [di, h, e]
    return np.ascontiguousarray(w.transpose(1, 0, 2).reshape(E, E))    # [h*D+di, e]


def kernel(x, y, Wqkv1, Wqkv2, Wout1, Wout2):
    x = np.asarray(x, dtype=np.float32)
    y = np.asarray(y, dtype=np.float32)

    if "nc" not in _NC_CACHE:
        _NC_CACHE["nc"] = _build()
    nc = _NC_CACHE["nc"]

    wq1_t = np.ascontiguousarray(_group_w(Wqkv1, 0).T)
    wk1_t = np.ascontiguousarray(_group_w(Wqkv1, 1).T)
    wv1_t = np.ascontiguousarray(_group_w(Wqkv1, 2).T)
    wq2_t = np.ascontiguousarray(_group_w(Wqkv2, 0).T)
    wk2_t = np.ascontiguousarray(_group_w(Wqkv2, 1).T)
    wv2_t = np.ascontiguousarray(_group_w(Wqkv2, 2).T)
    wout1_t = np.ascontiguousarray(np.asarray(Wout1, dtype=np.float32).T)
    wout2_t = np.ascontiguousarray(np.asarray(Wout2, dtype=np.float32).T)

    in_maps = []
    for c in range(N_CORES):
        s, b = divmod(c, B)
        if s == 0:
            # stream-1 output: K,V from x via Wqkv1; Q from y via Wqkv2
            a_t, b_t = x[b].T, y[b].T
            wq, wk, wv, wo = wq2_t, wk1_t, wv1_t, wout1_t
        else:
            a_t, b_t = y[b].T, x[b].T
            wq, wk, wv, wo = wq1_t, wk2_t, wv2_t, wout2_t
        in_maps.append({
            "a_t": np.ascontiguousarray(a_t).astype(np.float16),
            "b_t": np.ascontiguousarray(b_t).astype(np.float16),
            "wq_t": wq.astype(np.float16), "wk_t": wk.astype(np.float16),
            "wv_t": wv.astype(np.float16), "wout_t": wo.astype(np.float16),
        })

    trace = os.environ.get("BASS_KERNEL_TRACE", "0") == "1"
    if trace:
        try:
            from antenv.axon_hooks import get_axon_ntff_profile_hook  # noqa: F401
        except ImportError:
            trace = False
    ncores = int(os.environ.get("KCORES", str(N_CORES)))
    r = bass_utils.run_bass_kernel_spmd(nc, in_maps[:ncores], core_ids=list(range(ncores)),
                                        trace=trace)
    LAST_RESULTS["exec_time_ns"] = r.exec_time_ns
    LAST_RESULTS["profile_json"] = r.profile_json

    out1 = np.stack([r.results[b]["z_t"].T for b in range(B)]).astype(np.float32)
    out2 = np.stack([r.results[B + b]["z_t"].T for b in range(B)]).astype(np.float32)
    return out1, out2


# revision 21
# speedup vs baseline: 1.8483x; 1.2252x over previous
"""Trainium2 Bass kernel for nn_MultiHeadCrossAttention (B=4, T=1024, E=1024, H=16).

Sharding: 8 fully independent shards, zero cross-core communication:
(output stream s, batch b) for s in {1,2}, b in 0..3. Stream-1 output
xo@Wout1 needs K,V from x and Q from y; stream-2 the reverse.

Per-core kernel (activations transposed, feature-on-partition):
  Preamble: V = A^T.T @ Wv^T (natural, with ones column per head for the
  rowsum trick); Q^T/K^T chunks 0,1.
  m-loop over 8 head pairs (hA=2m, hB=2m+1), 16 slots (jc, ic) each:
    S^T pair via two concurrent K=64 matmuls (tile_position row split)
    P = exp(S/8) on ACT into SBUF fp16 (A|B merged per slot)
    O'^T accumulation (M=65 incl. ones row -> rowsum) lagged one slot
    Q^T/K^T projection chunk m+2 interleaved (2 matmuls per slot)
    normalization of pair m-1 lagged: reciprocal_approx_fast + gpsimd
    partition_broadcast + fp16 multiply into ot
  Tail: Z^T = Wout^T.T @ O^T accumulated over head pairs, DMA out.
"""

import os
import sys

sys.path.insert(0, "/opt/trn_rl_repo")

import numpy as np
import ml_dtypes
from contextlib import ExitStack

import concourse.bass as bass
import concourse.mybir as mybir
import concourse.tile as tile
from concourse import bacc
from concourse import bass_utils

B, T, E, H = 4, 1024, 1024, 16
D = E // H            # 64
NC = E // 128         # 8 chunks of 128
N_CORES = 8

F32 = mybir.dt.float32
F16 = mybir.dt.float16

_NC_CACHE = {}
LAST_RESULTS = {}
_KDBG = os.environ.get("KDBG", "0") == "1"
_KDBG2 = os.environ.get("KDBG", "0") == "2"
_DBG_TILES = {}


def _build():
    nc = bacc.Bacc("TRN2", target_bir_lowering=False, debug=False,
                   enable_asserts=False, num_devices=N_CORES)
    a_t = nc.dram_tensor("a_t", (E, T), F16, kind="ExternalInput").ap()
    b_t = nc.dram_tensor("b_t", (E, T), F16, kind="ExternalInput").ap()
    wq_t = nc.dram_tensor("wq_t", (E, E), F16, kind="ExternalInput").ap()
    wk_t = nc.dram_tensor("wk_t", (E, E), F16, kind="ExternalInput").ap()
    wv_t = nc.dram_tensor("wv_t", (E, E), F16, kind="ExternalInput").ap()
    wout_t = nc.dram_tensor("wout_t", (E, E), F16, kind="ExternalInput").ap()
    z_t = nc.dram_tensor("z_t", (E, T), F32, kind="ExternalOutput").ap()

    EXP = mybir.ActivationFunctionType.Exp

    with tile.TileContext(nc) as tc, ExitStack() as ctx:
        persist = ctx.enter_context(tc.tile_pool(name="persist", bufs=1))
        qt = persist.tile([128, NC, T], F16, tag="qt")
        kt = persist.tile([128, NC, T], F16, tag="kt")
        v = persist.tile([128, NC, H * (D + 1)], F16, tag="v")
        ot = persist.tile([128, NC, T], F16, tag="ot")
        wo_sb = None if _KDBG else persist.tile([128, NC, E], F16, tag="wo", name="wo_sb")

        for mch in range(NC):
            nc.vector.memset(
                v[:, mch, :].rearrange("p (h x) -> p h x", x=D + 1)[:, :, D:D + 1],
                1.0)
        if _KDBG:
            _DBG_TILES["zd"] = persist.tile([128, 6, T], F32, tag="zd", name="zd")
            nc.vector.memset(_DBG_TILES["zd"][:], 0.0)

        acts = ctx.enter_context(tc.tile_pool(name="acts", bufs=1))
        at_sb = acts.tile([128, NC, T], F16, tag="at")
        bt_sb = acts.tile([128, NC, T], F16, tag="bt")
        wq_sb = acts.tile([128, NC, E], F16, tag="wq")
        wk_sb = acts.tile([128, NC, E], F16, tag="wk")

        # ---------------- Preamble: V proj + QK chunks 0,1 ----------------
        with tc.tile_pool(name="wvp", bufs=1) as wvp, \
             tc.tile_pool(name="pps", bufs=2, space="PSUM") as pps:
            wv_sb = wvp.tile([128, NC, E], F16, tag="wv")
            # DMA priority: (at, wv) pairs first so V proj starts ASAP
            for e in range(NC):
                nc.sync.dma_start(at_sb[:, e, :], a_t[e * 128:(e + 1) * 128, :])
                nc.sync.dma_start(wv_sb[:, e, :], wv_t[e * 128:(e + 1) * 128, :])
            for e in range(NC):
                nc.sync.dma_start(bt_sb[:, e, :], b_t[e * 128:(e + 1) * 128, :])
                nc.sync.dma_start(wq_sb[:, e, :], wq_t[e * 128:(e + 1) * 128, :])
            for e in range(NC):
                nc.sync.dma_start(wk_sb[:, e, :], wk_t[e * 128:(e + 1) * 128, :])
            for e in range(NC) if not _KDBG else ():
                nc.sync.dma_start(wo_sb[:, e, :], wout_t[e * 128:(e + 1) * 128, :])

            # V natural: out[j-chunk m][h*d] = sum_e at[e, j].T @ wv[e, hd]
            for mch in range(NC):
                ps = pps.tile([128, T], F32, tag="pp")
                for e in range(NC):
                    for ic in range(2):
                        nc.tensor.matmul(
                            ps[:, bass.ts(ic, 512)],
                            at_sb[:, e, bass.ts(mch, 128)],
                            wv_sb[:, e, bass.ts(ic, 512)],
                            start=(e == 0), stop=(e == NC - 1))
                with nc.allow_low_precision(reason="V fp16 feeds fp16 matmul"):
                    nc.vector.tensor_copy(
                        v[:, mch, :].rearrange("p (h x) -> p h x", x=D + 1)[:, :, 0:D],
                        ps[:].rearrange("p (h d) -> p h d", d=D))

            # Q^T/K^T chunks 0 and 1
            for ch in (0, 1):
                for (w_sb, act_sb, out_sb) in ((wq_sb, bt_sb, qt), (wk_sb, at_sb, kt)):
                    ps = pps.tile([128, T], F32, tag="pp")
                    for e in range(NC):
                        for ic in range(2):
                            nc.tensor.matmul(
                                ps[:, bass.ts(ic, 512)],
                                w_sb[:, e, bass.ts(ch, 128)],
                                act_sb[:, e, bass.ts(ic, 512)],
                                start=(e == 0), stop=(e == NC - 1))
                    with nc.allow_low_precision(reason="QK fp16 feeds fp16 matmul"):
                        nc.vector.tensor_copy(out_sb[:, ch, :], ps[:])

        # ---------------- m-loop: attention over 8 head pairs ----------------
        with tc.tile_pool(name="sps", bufs=2, space="PSUM") as sps_pool, \
             tc.tile_pool(name="ops", bufs=1, space="PSUM") as ops, \
             tc.tile_pool(name="pjp", bufs=1, space="PSUM") as pjp, \
             tc.tile_pool(name="ptp", bufs=4) as ptp, \
             tc.tile_pool(name="oup", bufs=2) as oup, \
             tc.tile_pool(name="nrm", bufs=2) as nrm:

            SLOTS = [(ic, jc) for ic in range(2) for jc in range(NC)]
            pending_o = None   # (pt_tile, jc, ic, ps_oA, ps_oB, hA, hB)
            pending_norm = None  # (m, ouA, ouB, rs)

            def issue_o(po):
                pt_prev, jc, psA, psB, hA, hB = po
                st = dict(start=(jc == 0), stop=(jc == NC - 1))
                nc.tensor.matmul(psA[:, :],
                                 v[:, jc, hA * (D + 1):(hA + 1) * (D + 1)],
                                 pt_prev[:, 0:512], **st)
                nc.tensor.matmul(psB[:, :],
                                 v[:, jc, hB * (D + 1):(hB + 1) * (D + 1)],
                                 pt_prev[:, 512:1024], **st)

            def issue_norm(pn):
                mm, ouA, ouB, rsA, rsB = pn
                rrA = nrm.tile([1, T], F32, tag="rrA", bufs=1)
                rrB = nrm.tile([1, T], F32, tag="rrB", bufs=1)
                nc.vector.reciprocal_approx_fast(rrA[:], rsA[:])
                nc.vector.reciprocal_approx_fast(rrB[:], rsB[:])
                rrhA = nrm.tile([1, T], F16, tag="rrhA", bufs=1)
                rrhB = nrm.tile([1, T], F16, tag="rrhB", bufs=1)
                with nc.allow_low_precision(reason="recip feeds fp16 multiply"):
                    nc.vector.tensor_copy(rrhA[:], rrA[:])
                    nc.vector.tensor_copy(rrhB[:], rrB[:])
                bcA = nrm.tile([64, T], F16, tag="bcA", bufs=1)
                bcB = nrm.tile([64, T], F16, tag="bcB", bufs=1)
                nc.gpsimd.partition_broadcast(bcA[:], rrhA[:])
                nc.gpsimd.partition_broadcast(bcB[:], rrhB[:])
                with nc.allow_low_precision(reason="O^T fp16 feeds fp16 out-proj"):
                    nc.vector.tensor_mul(ot[0:64, mm, :], ouA[:], bcA[:])
                    nc.vector.tensor_mul(ot[64:128, mm, :], ouB[:], bcB[:])
                if _KDBG and mm == 0:
                    zd = _DBG_TILES["zd"]
                    nc.vector.tensor_copy(zd[0:64, 0, :], ouB[:])
                    nc.vector.tensor_copy(zd[0:64, 1, :], bcB[:])
                    nc.vector.tensor_copy(zd[0:1, 2, :], rsB[:])
                    nc.vector.tensor_copy(zd[32:33, 2, :], rrB[:])
                    nc.vector.tensor_copy(zd[64:65, 2, :], rrhB[:])

            def evac_half(psA, psB, ouA, ouB, rsA, rsB, ic):
                sl = bass.ts(ic, 512)
                with nc.allow_low_precision(reason="O' fp16 feeds fp16 multiply"):
                    nc.vector.tensor_copy(ouA[:, sl], psA[0:D, :])
                    nc.vector.tensor_copy(ouB[:, sl], psB[0:D, :])
                nc.vector.tensor_copy(rsA[:, sl], psA[D:D + 1, :])
                nc.vector.tensor_copy(rsB[:, sl], psB[D:D + 1, :])

            for m in range(NC):
                hA, hB = 2 * m, 2 * m + 1
                ps_oA = ops.tile([D + 1, 512], F32, tag="oA")
                ps_oB = ops.tile([D + 1, 512], F32, tag="oB")
                ouA = oup.tile([D, T], F16, tag="ouA")
                ouB = oup.tile([D, T], F16, tag="ouB")
                rsA = nrm.tile([1, T], F32, tag="rsA", bufs=1)
                rsB = nrm.tile([1, T], F32, tag="rsB", bufs=1)
                if pending_norm is not None:
                    issue_norm(pending_norm)
                    pending_norm = None

                pj = None
                for s, (ic, jc) in enumerate(SLOTS):
                    # S pair: two concurrent K=64 matmuls (row-split)
                    sps = sps_pool.tile([128, 1024], F32, tag="s")
                    nc.tensor.matmul(
                        sps[:, 0:512],
                        kt[0:64, m, bass.ts(jc, 128)],
                        qt[0:64, m, bass.ts(ic, 512)],
                        start=True, stop=True)
                    nc.tensor.matmul(
                        sps[:, 512:1024],
                        kt[64:128, m, bass.ts(jc, 128)],
                        qt[64:128, m, bass.ts(ic, 512)],
                        start=True, stop=True, tile_position=(64, 0))
                    pt_t = ptp.tile([128, 1024], F16, tag="pt")
                    nc.scalar.activation(pt_t[:], sps[:], EXP, scale=0.125)
                    if _KDBG and m == 0 and s == 0:
                        nc.vector.tensor_copy(_DBG_TILES["zd"][:, 4, :], pt_t[:])

                    if pending_o is not None:
                        issue_o(pending_o)
                    if s == 8:
                        evac_half(ps_oA, ps_oB, ouA, ouB, rsA, rsB, 0)
                        ps_oA = ops.tile([D + 1, 512], F32, tag="oA")
                        ps_oB = ops.tile([D + 1, 512], F32, tag="oB")
                    pending_o = (pt_t, jc, ps_oA, ps_oB, hA, hB)

                    # interleaved Q^T/K^T projection for chunk m+2
                    if m < NC - 2:
                        ch = m + 2
                        if s < 8:
                            e = s
                            if pj is None:
                                pj = pjp.tile([128, T], F32, tag="pj")
                            for icc in range(2):
                                nc.tensor.matmul(
                                    pj[:, bass.ts(icc, 512)],
                                    wq_sb[:, e, bass.ts(ch, 128)],
                                    bt_sb[:, e, bass.ts(icc, 512)],
                                    start=(e == 0), stop=(e == NC - 1))
                            if s == 7:
                                with nc.allow_low_precision(reason="QK fp16"):
                                    nc.vector.tensor_copy(qt[:, ch, :], pj[:])
                                pj = None
                        else:
                            e = s - 8
                            if pj is None:
                                pj = pjp.tile([128, T], F32, tag="pj")
                            for icc in range(2):
                                nc.tensor.matmul(
                                    pj[:, bass.ts(icc, 512)],
                                    wk_sb[:, e, bass.ts(ch, 128)],
                                    at_sb[:, e, bass.ts(icc, 512)],
                                    start=(e == 0), stop=(e == NC - 1))
                            if s == 15:
                                with nc.allow_low_precision(reason="QK fp16"):
                                    nc.vector.tensor_copy(kt[:, ch, :], pj[:])
                                pj = None

                # flush last O slot of this pair, then evacuate half 1
                issue_o(pending_o)
                pending_o = None
                evac_half(ps_oA, ps_oB, ouA, ouB, rsA, rsB, 1)
                pending_norm = (m, ouA, ouB, rsA, rsB)

            issue_norm(pending_norm)
            pending_norm = None

        if _KDBG:
            with tc.tile_pool(name="zdbg2", bufs=1) as zp2:
                zd = _DBG_TILES["zd"]
                nc.vector.tensor_copy(zd[0:64, 3, :], ot[0:64, 0, :])
                nc.vector.tensor_copy(zd[64:128, 3, :], ot[64:128, 0, :])
                nc.vector.tensor_copy(zd[:, 5, :], qt[:, 2, :])
                for cc in range(6):
                    nc.sync.dma_start(z_t[cc * 128:(cc + 1) * 128, :],
                                      zd[:, cc, :])

        if _KDBG2:
            with tc.tile_pool(name="zdbg3", bufs=2) as zp3:
                for mm in range(NC):
                    zc = zp3.tile([128, T], F32, tag="zc", name="zc")
                    nc.vector.tensor_copy(zc[:], ot[:, mm, :])
                    nc.sync.dma_start(z_t[mm * 128:(mm + 1) * 128, :], zc[:])

        # ---------------- Z: out-projection ----------------
        if not _KDBG and not _KDBG2:
          with tc.tile_pool(name="zps", bufs=2, space="PSUM") as zps, \
             tc.tile_pool(name="zsb", bufs=2) as zsbp:
            for cc in range(NC):
                ps = zps.tile([128, T], F32, tag="z")
                for mm in range(NC):
                    for ic in range(2):
                        nc.tensor.matmul(
                            ps[:, bass.ts(ic, 512)],
                            wo_sb[:, mm, bass.ts(cc, 128)],
                            ot[:, mm, bass.ts(ic, 512)],
                            start=(mm == 0), stop=(mm == NC - 1))
                zsb = zsbp.tile([128, T], F32, tag="zsb")
                nc.vector.tensor_copy(zsb[:], ps[:])
                nc.sync.dma_start(z_t[cc * 128:(cc + 1) * 128, :], zsb[:])
    nc.compile()
    return nc


def _group_w(wqkv, k):
    """Rows of Wqkv (3E, E) for q/k/v (k=0/1/2), grouped head-major.

    Row index layout: r = di*(3H) + k*H + h  ->  grouped[h*D+di, :].
    """
    w = np.asarray(wqkv, dtype=np.float32).reshape(D, 3, H, E)[:, k]   # [di, h, e]
    return np.ascontiguousarray(w.transpose(1, 0, 2).reshape(E, E))    # [h*D+di, e]


def kernel(x, y, Wqkv1, Wqkv2, Wout1, Wout2):
    x = np.asarray(x, dtype=np.float32)
    y = np.asarray(y, dtype=np.float32)

    if "nc" not in _NC_CACHE:
        _NC_CACHE["nc"] = _build()
    nc = _NC_CACHE["nc"]

    wq1_t = np.ascontiguousarray(_group_w(Wqkv1, 0).T)
    wk1_t = np.ascontiguousarray(_group_w(Wqkv1, 1).T)
    wv1_t = np.ascontiguousarray(_group_w(Wqkv1, 2).T)
    wq2_t = np.ascontiguousarray(_group_w(Wqkv2, 0).T)
    wk2_t = np.ascontiguousarray(_group_w(Wqkv2, 1).T)
    wv2_t = np.ascontiguousarray(_group_w(Wqkv2, 2).T)
    wout1_t = np.ascontiguousarray(np.asarray(Wout1, dtype=np.float32).T)
    wout2_t = np.ascontiguousarray(np.asarray(Wout2, dtype=np.float32).T)

    in_maps = []
    for c in range(N_CORES):
        s, b = divmod(c, B)
        if s == 0:
            # stream-1 output: K,V from x via Wqkv1; Q from y via Wqkv2
            a_t, b_t = x[b].T, y[b].T
            wq, wk, wv, wo = wq2_t, wk1_t, wv1_t, wout1_t
        else:
            a_t, b_t = y[b].T, x[b].T
            wq, wk, wv, wo = wq1_t, wk2_t, wv2_t, wout2_t
        in_maps.append({
            "a_t": np.ascontiguousarray(a_t).astype(np.float16),
            "b_t": np.ascontiguousarray(b_t).astype(np.float16),
            "wq_t": wq.astype(np.float16), "wk_t": wk.astype(np.float16),
            "wv_t": wv.astype(np.float16), "wout_t": wo.astype(np.float16),
        })

    trace = os.environ.get("BASS_KERNEL_TRACE", "0") == "1"
    if trace:
        try:
            from antenv.axon_hooks import get_axon_ntff_profile_hook  # noqa: F401
        except ImportError:
            trace = False
    ncores = int(os.environ.get("KCORES", str(N_CORES)))
    r = bass_utils.run_bass_kernel_spmd(nc, in_maps[:ncores], core_ids=list(range(ncores)),
                                        trace=trace)
    LAST_RESULTS["exec_time_ns"] = r.exec_time_ns
    LAST_RESULTS["profile_json"] = r.profile_json

    out1 = np.stack([r.results[b]["z_t"].T for b in range(B)]).astype(np.float32)
    out2 = np.stack([r.results[B + b]["z_t"].T for b in range(B)]).astype(np.float32)
    return out1, out2
